# revision 13
# baseline (speedup 1.0000x reference)
"""CGNN layer kernel for Trainium2 (8 NeuronCores, SPMD) — v4.

Sharding: core c owns batch b = c//2 and receiver-node half i0 = (c%2)*128.

Host-side prep (layout only):
  - j-axis compaction: per batch, gather the live sender columns (mask==1)
    and pad to a common NJC (multiple of 8). Padded columns are zero; the
    on-device korr correction (which removes silu(bias) pollution from
    zeroed columns) covers them via the shipped 0/1 maskf.
  - adj is pre-transposed to the PE-ready stack layout
    stk[(g r), q, j] = adj[i0 + 4q + g, j, r], masked, scaled 1/SD, fp8.
  - x^T masked/scaled/fp8 for the x_j term; xi^T fp32 for the ACb term.
  - W1 is split and packed into 4 DoubleRow lhsT variants
    L_g = [w1bT*SW ; Z_g] fp8 where Z_g has W1dT*SD at partition band g.
  - all small fp32 [H,H] consts ride in ONE packed DRAM param (one DMA);
    bias rows in another.

Device math (per core, b fixed):
  z[i] (h=128, j=NJC) = ONE fp8 DoubleRow matmul:
      ktile0: (W1b*SW)^T @ (x^T*mask/SW)   [K=128]
      ktile1: Z_g^T @ stack_q              [K=128, band-selected adj term]
  silu + per-receiver bias ACb[:,i]: ONE ACT op -> bf16 sink slice;
  per-quad segmented DVE tensor_reduce sums 4 receivers over j at once.
  ACb = W1a x_i + W1c c + b1 (fp32 matmuls, setup).
  S -= npad_or_dead * silu(ACb); aggr = W2 S + b2*live; update MLP (bf16
  matmuls) + LayerNorm epilogue.
"""

import numpy as np
import ml_dtypes
from contextlib import ExitStack

import concourse.bass as bass
import concourse.bacc as bacc
import concourse.mybir as mybir
import concourse.tile as tile
from concourse.bass_utils import run_bass_kernel_spmd

ml_bf16 = ml_dtypes.bfloat16
ml_f8 = ml_dtypes.float8_e4m3

B, N, H, R = 4, 256, 128, 32
NI = 128          # receivers per core
NQ = NI // 4      # receiver quads
FP = mybir.dt.float32
BF = mybir.dt.bfloat16
F8 = mybir.dt.float8e4
EPS = 1e-5
ALU = mybir.AluOpType
ACTF = mybir.ActivationFunctionType
DR = mybir.MatmulPerfMode.DoubleRow

SW = 8.0   # fp8 scale for the W1b / x^T k-tile
SD = 8.0   # fp8 scale for the W1d / adj k-tile

# packed fp32 [H,H] const slots
PKC = ["w1aT", "w1cT0", "w1cT1", "condrep0", "condrep1"]
PKO = ["identp", "gamma_rep", "beta_rep"]
# packed bf16 [H,H] const slots (epilogue matmul weights)
PKB = ["w2T", "w3aT", "w3bT", "w4T"]

_cache = {}


def _build_program(NJC):
    nc = bacc.Bacc()

    # ---- per-core DRAM parameters ----
    adj_stk = nc.declare_dram_parameter("adj_stk", [H, NQ, NJC], F8,
                                        isOutput=False)
    xT8 = nc.declare_dram_parameter("xT8", [H, NJC], F8, isOutput=False)
    xiT = nc.declare_dram_parameter("xiT", [H, NI], FP, isOutput=False)
    maskf = nc.declare_dram_parameter("maskf", [NJC], FP, isOutput=False)
    lhs8 = nc.declare_dram_parameter("lhs8", [H, 4, 2, H], F8, isOutput=False)
    packc = nc.declare_dram_parameter("packc", [H, len(PKC), H], FP,
                                      isOutput=False)
    packo = nc.declare_dram_parameter("packo", [H, len(PKO), H], FP,
                                      isOutput=False)
    packb = nc.declare_dram_parameter("packb", [H, len(PKB), H], BF,
                                      isOutput=False)
    rows = nc.declare_dram_parameter("rows", [1, 6, H], FP, isOutput=False)
    rowsb = nc.declare_dram_parameter("rowsb", [1, 5, H], BF, isOutput=False)
    out = nc.declare_dram_parameter("out", [NI, H], FP, isOutput=True)

    with ExitStack() as ctx:
        tc = ctx.enter_context(tile.TileContext(nc))
        const = ctx.enter_context(tc.tile_pool(name="const", bufs=1))
        persist = ctx.enter_context(tc.tile_pool(name="persist", bufs=1))
        work = ctx.enter_context(tc.tile_pool(name="work", bufs=2))
        scr = ctx.enter_context(tc.tile_pool(name="scr", bufs=4))
        pep = ctx.enter_context(tc.tile_pool(name="pep", bufs=2,
                                             space="PSUM"))
        pz = ctx.enter_context(tc.tile_pool(name="pz", bufs=6, space="PSUM"))

        # rhs "big" tile: slot 0 = x^T (masked, /SW, fp8); slots 1..NQ = adj
        # stacks, DMA'd directly from host-prepped DRAM. Issue these FIRST
        # (they gate the PE main loop), split across queues.
        rhsbig = persist.tile([H, NQ + 1, NJC], F8, tag="rhsbig",
                              name="rhsbig")
        # ACb-critical loads first (small, gate the ACT pipeline)
        xiT_sb = const.tile([H, NI], FP, tag="xiT", name="xiT")
        nc.sync.dma_start(out=xiT_sb, in_=xiT[:])
        packc_sb = const.tile([H, len(PKC), H], FP, tag="packc",
                              name="packc")
        nc.scalar.dma_start(out=packc_sb, in_=packc[:])
        rows_sb = const.tile([1, 6, H], FP, tag="rows", name="rows")
        nc.scalar.dma_start(out=rows_sb, in_=rows[:])
        lhs8_sb = const.tile([H, 4, 2, H], F8, tag="lhs8", name="lhs8")
        nc.gpsimd.dma_start(out=lhs8_sb, in_=lhs8[:])

        nc.sync.dma_start(out=rhsbig[:, 0], in_=xT8[:])
        CH = NQ // 8
        for ci in range(8):
            eng = nc.sync if ci % 2 == 0 else nc.gpsimd
            eng.dma_start(
                out=rhsbig[:, 1 + ci * CH:1 + (ci + 1) * CH],
                in_=adj_stk[:, ci * CH:(ci + 1) * CH])

        packb_sb = const.tile([H, len(PKB), H], BF, tag="packb", name="packb")
        nc.sync.dma_start(out=packb_sb, in_=packb[:])
        rowsb_sb = const.tile([1, 5, H], BF, tag="rowsb", name="rowsb")
        nc.gpsimd.dma_start(out=rowsb_sb, in_=rowsb[:])

        packo_sb = const.tile([H, len(PKO), H], FP, tag="packo",
                              name="packo")
        nc.scalar.dma_start(out=packo_sb, in_=packo[:])
        pk = {name: packc_sb[:, i] for i, name in enumerate(PKC)}
        pk.update({name: packo_sb[:, i] for i, name in enumerate(PKO)})
        pkb = {name: packb_sb[:, i] for i, name in enumerate(PKB)}
        b1r = rows_sb[0:1, 0]
        ones_r = rows_sb[0:1, 4]
        eps_r = rows_sb[0:1, 5]
        b2rb = rowsb_sb[0:1, 1]
        b3rb = rowsb_sb[0:1, 2]
        b4rb = rowsb_sb[0:1, 3]
        onesb_r = rowsb_sb[0:1, 4]

        # mask broadcast to all partitions: (128, NJC)
        maskrep = persist.tile([H, NJC], FP, tag="maskrep", name="maskrep")
        maskf_ap = maskf[:]
        mask_bcast = bass.AP(tensor=maskf_ap.tensor, offset=maskf_ap.offset,
                             ap=[[0, H]] + list(maskf_ap.ap))
        nc.gpsimd.dma_start(out=maskrep, in_=mask_bcast)

        # Preload the Silu ACT table early (hidden under DMA) with a dummy
        # op on the eps row (already-loaded const).
        dummy = work.tile([1, H], FP, tag="dummy", name="dummy")
        nc.scalar.activation(dummy, eps_r, ACTF.Silu)

        # per-partition live-count and dead/pad-count of sender slots
        msum = persist.tile([H, 1], FP, tag="msum", name="msum")
        mrow_scr = persist.tile([H, NJC], FP, tag="mrow_scr", name="mrow_scr")
        nc.vector.tensor_scalar(mrow_scr, maskrep, 1.0, None,
                                ALU.mult, ALU.add, accum_out=msum)
        nm0col = persist.tile([H, 1], FP, tag="nm0col", name="nm0col")
        nc.vector.tensor_scalar(nm0col, msum, -1.0, float(NJC),
                                ALU.mult, ALU.add)
        msum_rowb = persist.tile([1, NI], BF, tag="msum_rowb",
                                 name="msum_rowb")
        nc.vector.tensor_scalar(msum_rowb, ones_r, msum[0:1, 0:1], None,
                                ALU.mult)

        ACb = persist.tile([H, NI], FP, tag="ACb", name="ACb")
        siluAC = persist.tile([H, NI], FP, tag="siluAC", name="siluAC")
        korr = persist.tile([H, NI], FP, tag="korr", name="korr")
        S_raw = persist.tile([H, NI], FP, tag="S_raw", name="S_raw")

        # ACb = W1a x_i + W1c c + b1  -> (128 h, 128 i)
        pA = pep.tile([H, NI], FP, tag="ps", name="pA")
        nc.tensor.matmul(pA, lhsT=pk["w1aT"], rhs=xiT_sb,
                         start=True, stop=False)
        nc.tensor.matmul(pA, lhsT=pk["w1cT0"], rhs=pk["condrep0"],
                         start=False, stop=False)
        nc.tensor.matmul(pA, lhsT=pk["w1cT1"], rhs=pk["condrep1"],
                         start=False, stop=False)
        nc.tensor.matmul(pA, lhsT=b1r, rhs=ones_r,
                         start=False, stop=True)
        nc.vector.tensor_copy(ACb, pA)

        # korr[h,i] = (dead+pad count) * silu(ACb[h,i])
        nc.scalar.activation(siluAC, ACb, ACTF.Silu)
        nc.vector.tensor_scalar(korr, siluAC, nm0col, None, ALU.mult)

        # bf16 x_i^T for the update-MLP matmuls (setup, persists)
        xiTb = persist.tile([H, NI], BF, tag="xiTb", name="xiTb")
        nc.gpsimd.tensor_copy(xiTb, xiT_sb)

        # x_i halves in row-major (partitions 0:64 each) for the residual
        xi_rows = []
        for hf in range(2):
            xr = persist.tile([64, H], FP, tag=f"xi_row{hf}",
                              name=f"xi_row{hf}")
            pxir = pep.tile([64, H], FP, tag="ps", name="pxir")
            nc.tensor.transpose(pxir, xiT_sb[:, 64 * hf:64 * hf + 64],
                                pk["identp"])
            nc.vector.tensor_copy(xr, pxir)
            xi_rows.append(xr)

        # ---- epilogue, split by receiver half for overlap ----
        ep = {}

        def phase_a(hf):
            sl = slice(64 * hf, 64 * hf + 64)
            ones64 = rowsb_sb[0:1, 4, 0:64]
            S_true = work.tile([H, 64], BF, tag="S_true", name="S_true")
            nc.vector.scalar_tensor_tensor(out=S_true, in0=S_raw[:, sl],
                                           scalar=0.0, in1=korr[:, sl],
                                           op0=ALU.add, op1=ALU.subtract)
            pa = pep.tile([H, 64], FP, tag="ps", name="pa")
            nc.tensor.matmul(pa, lhsT=pkb["w2T"], rhs=S_true,
                             start=True, stop=False)
            nc.tensor.matmul(pa, lhsT=b2rb, rhs=msum_rowb[0:1, sl],
                             start=False, stop=True)
            aggrT = work.tile([H, 64], BF, tag="aggrT", name="aggrT")
            nc.vector.tensor_copy(aggrT, pa)
            pu = pep.tile([H, 64], FP, tag="ps", name="pu")
            nc.tensor.matmul(pu, lhsT=pkb["w3aT"], rhs=xiTb[:, sl],
                             start=True, stop=False)
            nc.tensor.matmul(pu, lhsT=pkb["w3bT"], rhs=aggrT,
                             start=False, stop=False)
            nc.tensor.matmul(pu, lhsT=b3rb, rhs=ones64, start=False,
                             stop=True)
            u_sb = work.tile([H, 64], BF, tag="u_sb", name="u_sb")
            nc.scalar.activation(u_sb, pu, ACTF.Silu)
            pupd = pep.tile([H, 64], FP, tag="ps", name="pupd")
            nc.tensor.matmul(pupd, lhsT=pkb["w4T"], rhs=u_sb,
                             start=True, stop=False)
            nc.tensor.matmul(pupd, lhsT=b4rb, rhs=ones64, start=False,
                             stop=True)
            updT = work.tile([H, 64], FP, tag="updT", name="updT")
            nc.vector.tensor_copy(updT, pupd)
            py = pep.tile([64, H], FP, tag="ps", name="py")
            nc.tensor.transpose(py, updT, pk["identp"])
            y_sb = work.tile([64, H], FP, tag="y_sb", name="y_sb")
            rowsum = work.tile([64, 1], FP, tag="rowsum", name="rowsum")
            nc.vector.scalar_tensor_tensor(out=y_sb, in0=py, scalar=0.0,
                                           in1=xi_rows[hf], op0=ALU.add,
                                           op1=ALU.add, accum_out=rowsum)
            ep[hf] = (y_sb, rowsum)

        def phase_bc(hf):
            y_sb, rowsum = ep[hf]
            sl = slice(64 * hf, 64 * hf + 64)
            negmu = work.tile([64, 1], FP, tag="negmu", name="negmu")
            nc.vector.tensor_scalar(negmu, rowsum, -1.0 / H, None, ALU.mult)
            ysq = work.tile([64, H], FP, tag="ysq", name="ysq")
            sumsq = work.tile([64, 1], FP, tag="sumsq", name="sumsq")
            nc.vector.scalar_tensor_tensor(out=ysq, in0=y_sb, scalar=0.0,
                                           in1=y_sb, op0=ALU.add,
                                           op1=ALU.mult, accum_out=sumsq)
            ex2 = work.tile([64, 1], FP, tag="ex2", name="ex2")
            nc.vector.tensor_scalar(ex2, sumsq, 1.0 / H, float(EPS),
                                    ALU.mult, ALU.add)
            musq = work.tile([64, 1], FP, tag="musq", name="musq")
            nc.vector.scalar_tensor_tensor(out=musq, in0=negmu, scalar=0.0,
                                           in1=negmu, op0=ALU.add,
                                           op1=ALU.mult)
            vare = work.tile([64, 1], FP, tag="vare", name="vare")
            nc.vector.scalar_tensor_tensor(out=vare, in0=ex2, scalar=0.0,
                                           in1=musq, op0=ALU.add,
                                           op1=ALU.subtract)
            sd = work.tile([64, 1], FP, tag="sd", name="sd")
            nc.scalar.activation(sd, vare, ACTF.Sqrt)
            rstd = work.tile([64, 1], FP, tag="rstd", name="rstd")
            nc.vector.reciprocal(rstd, sd)
            yn = work.tile([64, H], FP, tag="yn", name="yn")
            nc.vector.tensor_scalar(yn, y_sb, negmu, rstd, ALU.add, ALU.mult)
            yg = work.tile([64, H], FP, tag="yg", name="yg")
            nc.vector.scalar_tensor_tensor(out=yg, in0=yn, scalar=0.0,
                                           in1=pk["gamma_rep"][0:64, :],
                                           op0=ALU.add, op1=ALU.mult)
            yfin = work.tile([64, H], FP, tag="yfin", name="yfin")
            nc.vector.scalar_tensor_tensor(out=yfin, in0=yg, scalar=0.0,
                                           in1=pk["beta_rep"][0:64, :],
                                           op0=ALU.add, op1=ALU.add)
            nc.sync.dma_start(out=out[sl], in_=yfin)


        # ---- main loop: one DoubleRow matmul + one ACT per receiver;
        # one segmented DVE reduce (sum over j, 4 receivers) per quad ----
        for q in range(NQ):
            rhs_q = rhsbig[:, 0:q + 2:q + 1]   # slots {0, q+1}
            sinkq = scr.tile([H, 4, NJC], BF, tag="sinkq", name="sinkq")
            for g in range(4):
                li = 4 * q + g
                zt = pz.tile([H, NJC], FP, tag="zt", name="zt")
                nc.tensor.matmul(zt, lhsT=lhs8_sb[:, g], rhs=rhs_q,
                                 start=True, stop=True, perf_mode=DR)
                nc.scalar.activation(sinkq[:, g], zt, ACTF.Silu,
                                     bias=ACb[:, li:li + 1])
            nc.vector.tensor_reduce(S_raw[:, 4 * q:4 * q + 4], sinkq,
                                    mybir.AxisListType.X, ALU.add)
            if q == 28:
                phase_a(0)

        phase_a(1)
        phase_bc(0)
        phase_bc(1)

    nc.finalize()
    return nc


def _get_program(NJC):
    key = ("nc", NJC)
    if key not in _cache:
        _cache[key] = _build_program(NJC)
    return _cache[key]


def kernel(x, adj_dist, mask, cond_vec, W1, b1, W2, b2, W3, b3, W4, b4,
           gamma, beta):
    x = np.asarray(x, dtype=np.float32)
    adj_dist = np.asarray(adj_dist, dtype=np.float32)
    mask_np = np.asarray(mask)
    cond_vec = np.asarray(cond_vec, dtype=np.float32)
    W1 = np.asarray(W1, dtype=np.float32)
    W2 = np.asarray(W2, dtype=np.float32)
    W3 = np.asarray(W3, dtype=np.float32)
    W4 = np.asarray(W4, dtype=np.float32)

    def c(a):
        return np.ascontiguousarray(a, dtype=np.float32)

    # j-axis compaction: live sender indices per batch, padded to common NJC
    live_idx = [np.where(mask_np[b] != 0)[0] for b in range(B)]
    NJC = max(8, -(-max(len(ix) for ix in live_idx) // 8) * 8)

    # DoubleRow lhsT: [H(k), 4(g), 2(ktile), H(m)] fp8
    w1bT = W1[:, H:2 * H].T               # (k, h)
    w1dT = W1[:, 2 * H:2 * H + R].T       # (r, h)
    lhs8 = np.zeros((H, 4, 2, H), dtype=np.float32)
    lhs8[:, :, 0, :] = (w1bT * SW)[:, None, :]
    for g in range(4):
        lhs8[32 * g:32 * g + 32, g, 1, :] = w1dT * SD
    lhs8 = lhs8.astype(ml_f8)

    rows_np = np.zeros((1, 6, H), dtype=np.float32)
    rows_np[0, 0] = np.asarray(b1)
    rows_np[0, 1] = np.asarray(b2)
    rows_np[0, 2] = np.asarray(b3)
    rows_np[0, 3] = np.asarray(b4)
    rows_np[0, 4] = 1.0
    rows_np[0, 5] = EPS

    rowsb_np = rows_np[:, :5].astype(ml_bf16)

    packb_np = np.stack([W2.T, W3[:, 0:H].T, W3[:, H:2 * H].T, W4.T],
                        axis=1).astype(ml_bf16)

    gamma_rep = np.tile(np.asarray(gamma, dtype=np.float32)[None, :], (H, 1))
    beta_rep = np.tile(np.asarray(beta, dtype=np.float32)[None, :], (H, 1))
    packo_np = np.stack([np.eye(H, dtype=np.float32), gamma_rep, beta_rep],
                        axis=1)
    shared = dict(
        packo=np.ascontiguousarray(packo_np),
        lhs8=lhs8,
        rows=rows_np,
        rowsb=rowsb_np,
        packb=np.ascontiguousarray(packb_np),
    )


    in_maps = []
    for core in range(8):
        b, ih = core // 2, core % 2
        i0 = ih * NI
        ix = live_idx[b]
        nlive = len(ix)

        # gathered x^T, masked (pad cols zero), scaled, fp8
        xg = np.zeros((H, NJC), dtype=np.float32)
        xg[:, :nlive] = x[b][ix].T / SW
        # adj stacks: [(g r), q, j] = adj[i0+4q+g, j_live, r] / SD
        ag = adj_dist[b, i0:i0 + NI][:, ix, :]          # (128, nlive, R)
        stk = np.zeros((H, NQ, NJC), dtype=np.float32)
        a4 = ag.reshape(NQ, 4, nlive, R)                # (q, g, j, r)
        stk[:, :, :nlive] = (a4.transpose(1, 3, 0, 2)   # (g, r, q, j)
                             .reshape(H, NQ, nlive)) / SD
        mf = np.zeros((NJC,), dtype=np.float32)
        mf[:nlive] = 1.0

        condrep = np.tile(cond_vec[b][:, None], (1, H)).astype(np.float32)
        packc_np = np.stack(
            [W1[:, 0:H].T, W1[:, 2 * H + R:3 * H + R].T,
             W1[:, 3 * H + R:].T, condrep[0:H], condrep[H:2 * H]], axis=1)

        m = dict(shared)
        m["adj_stk"] = stk.astype(ml_f8)
        m["xT8"] = xg.astype(ml_f8)
        m["xiT"] = c(x[b, i0:i0 + NI].T)
        m["maskf"] = mf
        m["packc"] = np.ascontiguousarray(packc_np)
        in_maps.append(m)

    nc = _get_program(NJC)
    _cache["in_maps"] = in_maps
    _cache["last_njc"] = NJC
    res = run_bass_kernel_spmd(nc, in_maps, list(range(8)))

    out_full = np.empty((B, N, H), dtype=np.float32)
    for core in range(8):
        b, ih = core // 2, core % 2
        out_full[b, ih * NI:(ih + 1) * NI] = res.results[core]["out"]
    return out_full
